# revision 15
# baseline (speedup 1.0000x reference)
"""Trainium2 Bass kernel v2 for nn_MessagePassing (GNN last-writer message passing).

Math (from the reference):
  src[l,j]  = max{ i : adj[l,i,j]==1 } U {j}          (last writer wins)
  deg[l,i]  = 1 + sum_j adj[l,i,j]
  out[j,l,:] = (W @ feature[src[l,j], l, :]) / sqrt(deg[l,src]*deg[l,j])

v2 strategy (vs v1 baseline):
  - Adjacency ships BIT-PACKED (2 MB/core in each of two layouts, vs 16 MB
    of fp8 bytes): pbt (dest-row-major, bits over source i) for src-finding,
    pbd (source-row-major, bits over the dest shard j) for degrees.
  - Degrees via SWAR popcount on VectorE (i32 shift/and/add passes), not
    1024 tiny PE matmuls.  Partials transposed on PE, AllReduced (128 KB).
  - src via segmented reduce_max over int32 words (last nonzero word), an
    is_equal+bitand+reduce_add word extraction, and a float32-exponent trick
    for the top set bit of the 32-bit word (exact via u16 halves).
  - All gathers collapse into TWO dma_gather calls (4096x512B features,
    8192x256B degree blocks) instead of 128 serial [P,1] indirect DMAs.
    Gather indices computed on-chip, bounced through DRAM into the wrapped
    [16, n/16] int16 layout dma_gather requires.
  - Degree values extracted from gathered 64-f32 blocks with iota==offset
    masks + reduce_add (offset = src mod 64 is data-dependent per partition).
"""

import os
import sys
import types
from contextlib import ExitStack

import numpy as np

for _p in ("/opt/trn_rl_repo",):
    if os.path.isdir(_p) and _p not in sys.path:
        sys.path.append(_p)

from concourse import bacc, bass, mybir
from concourse.masks import make_identity
from concourse.tile import TileContext

F32 = mybir.dt.float32
I32 = mybir.dt.int32
I16 = mybir.dt.int16
U8 = mybir.dt.uint8
U16 = mybir.dt.uint16
AX = mybir.AxisListType
OP = mybir.AluOpType
ACT = mybir.ActivationFunctionType

P = 128  # SBUF partitions

N_NODES, N_LAYERS, D, N_CORES = 4096, 8, 128, 8


def _install_ntff_hook():
    """This image's antenv lacks axon_hooks; synthesize it so trace=True works."""
    try:
        import antenv
    except ImportError:
        return
    if "antenv.axon_hooks" in sys.modules:
        return
    mod = types.ModuleType("antenv.axon_hooks")
    _state = {"hook": None}
    mod.set_axon_ntff_profile_hook = lambda h: _state.__setitem__("hook", h)
    mod.get_axon_ntff_profile_hook = lambda: _state["hook"]
    sys.modules["antenv.axon_hooks"] = mod
    antenv.axon_hooks = mod
    try:
        from trn_agent_boot.trn_boot import _ntff_profile_via_ctypes

        mod.set_axon_ntff_profile_hook(
            _ntff_profile_via_ctypes("/opt/axon/libaxon_pjrt.so")
        )
    except Exception:
        pass


def build_kernel(N=N_NODES, L=N_LAYERS, n_cores=N_CORES, debug=False):
    """One SPMD program, identical on all cores; per-core data arrives via inputs."""
    JJ = N // n_cores  # dests per core
    T = JJ // P  # 128-row dest tiles per layer
    G = L * T  # (layer, tile) groups
    WR = N // 32  # i32 words per pbt row (src bits)
    WD = JJ // 32  # i32 words per pbd row (dest-shard bits)
    NB = L * (N // P)  # (l, src-block) segments in pbd image
    DBLK = 64  # f32 per gathered degree block (256 B)
    NI = G * P  # feature gather count
    assert N * L <= 2**15, "gather indices must fit int16"
    assert NB % P == 0

    nc = bacc.Bacc()
    pbt = nc.declare_dram_parameter("pbt", [P, G * WR], I32, isOutput=False)
    pbd = nc.declare_dram_parameter("pbd", [P, NB * WD], I32, isOutput=False)
    featg = nc.declare_dram_parameter("featg", [N * L, D], F32, isOutput=False)
    wt = nc.declare_dram_parameter("wt", [D, D], F32, isOutput=False)
    iotw = nc.declare_dram_parameter("iotw", [P, G * WR], F32, isOutput=False)
    jg = nc.declare_dram_parameter("jg", [P, G], F32, isOutput=False)
    ladd = nc.declare_dram_parameter("ladd", [P, G], F32, isOutput=False)
    lN = nc.declare_dram_parameter("lN", [P, G], F32, isOutput=False)
    djofs = nc.declare_dram_parameter("djofs", [8, 1], I32, isOutput=False)
    out = nc.declare_dram_parameter("out", [JJ, L, D], F32, isOutput=True)

    if debug:
        o_src = nc.declare_dram_parameter("o_src", [P, G], F32, isOutput=True)
        o_degp = nc.declare_dram_parameter("o_degp", [P, NB], F32, isOutput=True)
        o_degs = nc.declare_dram_parameter("o_degs", [P, G], F32, isOutput=True)
        o_degj = nc.declare_dram_parameter("o_degj", [P, G], F32, isOutput=True)
        o_rsc = nc.declare_dram_parameter("o_rsc", [P, G], F32, isOutput=True)
        o_feat = nc.declare_dram_parameter("o_feat", [P, G * D], F32, isOutput=True)
    deg_in = nc.dram_tensor("deg_in", [NB, P], F32)
    HR = L * N // DBLK // 2
    deg_out1 = nc.dram_tensor("deg_out1", [HR, DBLK], F32, addr_space="Shared")
    deg_out2 = nc.dram_tensor("deg_out2", [HR, DBLK], F32, addr_space="Shared")


    with TileContext(nc) as tc, ExitStack() as ctx:
        const = ctx.enter_context(tc.tile_pool(name="const", bufs=1))
        big = ctx.enter_context(tc.tile_pool(name="big", bufs=1))
        sml = ctx.enter_context(tc.tile_pool(name="sml", bufs=1))
        mm = ctx.enter_context(tc.tile_pool(name="mm", bufs=3))
        psum = ctx.enter_context(tc.tile_pool(name="psum", bufs=2, space="PSUM"))
        psum2 = ctx.enter_context(tc.tile_pool(name="psum2", bufs=2, space="PSUM"))

        # ---- adjacency bit images first: they gate the critical path ----
        pbd_sb = big.tile([P, NB * WD], I32, tag="pbd")
        nc.sync.dma_start(pbd_sb[:, 0 : NB * WD // 2], pbd.ap()[:, 0 : NB * WD // 2])
        nc.sync.dma_start(
            pbd_sb[:, NB * WD // 2 : NB * WD], pbd.ap()[:, NB * WD // 2 : NB * WD]
        )
        pbt_sb = big.tile([P, G * WR], I32, tag="pbt")
        nc.sync.dma_start(pbt_sb[:], pbt.ap())
        iotw_sb = const.tile([P, G * WR], F32)
        nc.sync.dma_start(iotw_sb[:], iotw.ap())

        # ---- constants ----
        eye = const.tile([P, P], F32)
        make_identity(nc, eye[:])
        wt_sb = const.tile([D, D], F32)
        nc.sync.dma_start(wt_sb[:], wt[:])
        jg_sb = const.tile([P, G], F32)
        nc.sync.dma_start(jg_sb[:], jg.ap())
        ladd_sb = const.tile([P, G], F32)
        nc.sync.dma_start(ladd_sb[:], ladd.ap())
        lN_sb = const.tile([P, G], F32)
        nc.sync.dma_start(lN_sb[:], lN.ap())
        djofs_sb = sml.tile([8, 1], I32, tag="djofs")
        nc.sync.dma_start(djofs_sb[:], djofs.ap())

        # ---- SWAR popcount degrees: partial deg over the dest shard ----
        # (DVE int add/sub/mult upcast to fp32 -> exact only below 2^24, so the
        # whole popcount runs in u16 lanes; bitwise/shift ops are bit-exact.)
        # Processed in two layer-halves, each feeding its own AllReduce so the
        # collective latency overlaps the rest of the degree/src pipeline.
        t1 = big.tile([P, NB * WD], I32, tag="t1")
        t2 = big.tile([P, NB * WD], I32, tag="t2")
        t3 = big.tile([P, NB * WD], I32, tag="t3")
        r2 = sml.tile([P, 2 * NB], I32, tag="r2")
        b0 = sml.tile([P, 2 * NB], I32, tag="b0")
        b1 = sml.tile([P, 2 * NB], I32, tag="b1")
        degf2 = sml.tile([P, 2 * NB], F32, tag="degf2")
        degf = sml.tile([P, NB], F32, tag="degf")
        HB = NB // 2  # segments per half
        for h in range(2):
            u = slice(h * HB * WD * 2, (h + 1) * HB * WD * 2)  # u16 cols
            vu = pbd_sb[:].bitcast(U16)[:, u]
            t1u = t1[:].bitcast(U16)[:, u]
            t2u = t2[:].bitcast(U16)[:, u]
            t3u = t3[:].bitcast(U16)[:, u]
            nc.vector.tensor_scalar(
                t1u, vu, 1, 0x5555, OP.logical_shift_right, OP.bitwise_and
            )
            nc.vector.tensor_tensor(t2u, vu, t1u, op=OP.subtract)
            nc.vector.tensor_scalar(t1u, t2u, 0x3333, None, OP.bitwise_and)
            nc.vector.tensor_scalar(
                t3u, t2u, 2, 0x3333, OP.logical_shift_right, OP.bitwise_and
            )
            nc.vector.tensor_tensor(t2u, t1u, t3u, op=OP.add)
            nc.vector.tensor_scalar(t1u, t2u, 4, None, OP.logical_shift_right)
            nc.vector.tensor_tensor(t2u, t2u, t1u, op=OP.add)
            nc.vector.tensor_scalar(t1u, t2u, 0x0F0F, None, OP.bitwise_and)
            # t1u byte lanes hold per-byte counts (<=8); sum half-rows of WD
            # u16 lanes (byte-lane partials <=128, no cross-lane carry)
            r2h = r2[:, 2 * h * HB : 2 * (h + 1) * HB]
            with nc.allow_low_precision(reason="exact small-int popcount"):
                nc.vector.tensor_reduce(
                    r2h,
                    t1u.rearrange("p (s w) -> p s w", w=WD),
                    axis=AX.X,
                    op=OP.add,
                )
            b0h = b0[:, 2 * h * HB : 2 * (h + 1) * HB]
            b1h = b1[:, 2 * h * HB : 2 * (h + 1) * HB]
            nc.vector.tensor_scalar(b0h, r2h, 0xFF, None, OP.bitwise_and)
            nc.vector.tensor_scalar(b1h, r2h, 8, None, OP.logical_shift_right)
            nc.vector.tensor_tensor(b0h, b0h, b1h, op=OP.add)
            dfh = degf2[:, 2 * h * HB : 2 * (h + 1) * HB]
            nc.vector.tensor_copy(dfh, b0h)
            degfh = degf[:, h * HB : (h + 1) * HB]
            nc.vector.tensor_reduce(
                degfh,
                dfh.rearrange("p (s two) -> p s two", two=2),
                axis=AX.X,
                op=OP.add,
            )
            # transpose to (l,b)-major rows, write partials, AllReduce the half
            dt = psum.tile([P, P], F32, tag="pt")
            nc.tensor.transpose(dt[0:HB, :], degfh, eye[:])
            dT = mm.tile([P, P], F32, tag="dT")
            nc.scalar.copy(dT[0:HB, :], dt[0:HB, :])
            nc.sync.dma_start(deg_in[h * HB : (h + 1) * HB, :], dT[0:HB, :])
            nc.gpsimd.collective_compute(
                "AllReduce",
                OP.add,
                ins=[deg_in[h * HB : (h + 1) * HB, :].opt()],
                outs=[(deg_out1 if h == 0 else deg_out2).ap().opt()],
                replica_groups=[list(range(n_cores))],
            )

        # ---- src finding on pbt: last nonzero word, then top set bit ----
        # (processed in two layer-halves so feature gathers can start while the
        # second half is still computing; reuses t1/t2 as scratch)
        Wp1 = sml.tile([P, G], F32, tag="Wp1")
        vhalf = sml.tile([P, 2 * G], I32, tag="vhalf")
        hi = sml.tile([P, G], I32, tag="hi")
        lo = sml.tile([P, G], I32, tag="lo")
        fhi = sml.tile([P, G], F32, tag="fhi")
        flo = sml.tile([P, G], F32, tag="flo")
        bp = sml.tile([P, G], I32, tag="bp")
        bpf = sml.tile([P, G], F32, tag="bpf")
        sa = sml.tile([P, G], F32, tag="sa")
        srcf = sml.tile([P, G], F32, tag="srcf")
        src = sml.tile([P, G], F32, tag="src")
        s8 = sml.tile([P, G], F32, tag="s8")
        fidxf = sml.tile([P, G], F32, tag="fidxf")
        didxf = sml.tile([P, G], F32, tag="didxf")
        didxi = sml.tile([P, G], I32, tag="didxi")
        fidxi = sml.tile([P, G], I32, tag="fidxi")

        def phase_c(ga, gb):
            gs = slice(ga, gb)
            cs = slice(ga * WR, gb * WR)
            w = pbt_sb[:, cs]
            t1f = t1[:, cs].bitcast(F32)
            t2f = t2[:, cs].bitcast(F32)
            nc.vector.tensor_scalar(t1f, w, 0, None, OP.not_equal)
            nc.vector.tensor_tensor(t2f, iotw_sb[:, cs], t1f, op=OP.mult)
            nc.vector.tensor_reduce(
                Wp1[:, gs],
                t2f.rearrange("p (g w) -> p g w", w=WR),
                axis=AX.X,
                op=OP.max,
            )
            for g in range(ga, gb):
                nc.vector.tensor_scalar(
                    t2[:, g * WR : (g + 1) * WR],
                    iotw_sb[:, g * WR : (g + 1) * WR],
                    Wp1[:, g : g + 1],
                    None,
                    OP.is_equal,
                )
            nc.vector.tensor_scalar(
                t1[:, cs], t2[:, cs], 31, 31, OP.logical_shift_left, OP.arith_shift_right
            )
            nc.vector.tensor_tensor(t2[:, cs], w, t1[:, cs], op=OP.bitwise_and)
            # one strided u16 reduce extracts both halves of the selected word
            # (fp32 accumulator exact for <=65535; a full i32 word is not)
            with nc.allow_low_precision(reason="exact u16-half one-hot extraction"):
                nc.vector.tensor_reduce(
                    vhalf[:, 2 * ga : 2 * gb].rearrange("p (g two) -> p g two", two=2),
                    t2[:, cs]
                    .bitcast(U16)
                    .rearrange("p (g w two) -> p g two w", two=2, w=WR),
                    axis=AX.X,
                    op=OP.add,
                )
            vh = vhalf[:, 2 * ga : 2 * gb].rearrange("p (g two) -> p g two", two=2)
            nc.vector.tensor_copy(flo[:, gs], vh[:, :, 0])
            nc.vector.tensor_copy(fhi[:, gs], vh[:, :, 1])
            # top set bit via the float32-exponent trick on each half
            # (-127/-32 offsets fold into sa's constant, +16 biases hi)
            nc.vector.tensor_scalar(
                hi[:, gs], fhi[:, gs].bitcast(I32), 23, None, OP.logical_shift_right
            )
            nc.vector.tensor_scalar(
                lo[:, gs], flo[:, gs].bitcast(I32), 23, None, OP.logical_shift_right
            )
            nc.vector.tensor_scalar(hi[:, gs], hi[:, gs], 16, None, OP.add)
            nc.vector.tensor_tensor(bp[:, gs], hi[:, gs], lo[:, gs], op=OP.max)
            nc.vector.tensor_copy(bpf[:, gs], bp[:, gs])
            nc.vector.tensor_scalar(sa[:, gs], Wp1[:, gs], 32.0, -159.0, OP.mult, OP.add)
            nc.vector.tensor_tensor(srcf[:, gs], sa[:, gs], bpf[:, gs], op=OP.add)
            nc.vector.tensor_tensor(src[:, gs], srcf[:, gs], jg_sb[:, gs], op=OP.max)
            nc.vector.tensor_scalar(s8[:, gs], src[:, gs], float(L), None, OP.mult)
            nc.vector.tensor_tensor(fidxf[:, gs], s8[:, gs], ladd_sb[:, gs], op=OP.add)
            nc.vector.tensor_tensor(didxf[:, gs], src[:, gs], lN_sb[:, gs], op=OP.add)
            nc.vector.tensor_copy(didxi[:, gs], didxf[:, gs])
            nc.vector.tensor_copy(fidxi[:, gs], fidxf[:, gs])

        # ---- gathers: per-group indirect DMAs, interleaved with phase C ----
        degs = sml.tile([P, G], F32, tag="degs")
        featsb = big.tile([P, G * D], F32, tag="featsb")
        df1 = deg_out1.ap().rearrange("r c -> (r c)").unsqueeze(1)
        df2 = deg_out2.ap().rearrange("r c -> (r c)").unsqueeze(1)

        def feat_gathers(ga, gb):
            for g in range(ga, gb):
                nc.gpsimd.indirect_dma_start(
                    out=featsb[:, g * D : (g + 1) * D],
                    out_offset=None,
                    in_=featg.ap(),
                    in_offset=bass.IndirectOffsetOnAxis(ap=fidxi[:, g : g + 1], axis=0),
                )

        def deg_gathers(ga, gb, df):
            for g in range(ga, gb):
                nc.gpsimd.indirect_dma_start(
                    out=degs[:, g : g + 1],
                    out_offset=None,
                    in_=df,
                    in_offset=bass.IndirectOffsetOnAxis(ap=didxi[:, g : g + 1], axis=0),
                )

        degj_raw0 = big.tile([4, JJ], F32, tag="degjr0")
        degj_raw1 = big.tile([4, JJ], F32, tag="degjr1")
        degj_raw = [degj_raw0, degj_raw1]

        def degj_gather(h):
            # own-j degrees: indirect fetch of 4x512 contiguous f32 rows
            # (deg_outN viewed [32, 512]; row (l%4)*8 + core)
            nc.gpsimd.indirect_dma_start(
                out=degj_raw[h][:],
                out_offset=None,
                in_=(deg_out1 if h == 0 else deg_out2)
                .ap()
                .rearrange("(a b) c -> a (b c)", b=JJ // DBLK),
                in_offset=bass.IndirectOffsetOnAxis(
                    ap=djofs_sb[4 * h : 4 * h + 4, 0:1], axis=0
                ),
            )

        phase_c(0, G // 2)
        feat_gathers(0, G // 2)
        deg_gathers(0, G // 2, df1)
        degj_gather(0)
        phase_c(G // 2, G)
        deg_gathers(G // 2, G, df2)
        degj_gather(1)
        feat_gathers(G // 2, G)

        if debug:
            nc.sync.dma_start(o_src.ap(), src[:])
            nc.sync.dma_start(o_degp.ap(), degf[:])
            nc.sync.dma_start(o_feat.ap(), featsb[:])

        # ---- feature transposes + unscaled W matmuls (run during gathers) ----
        gts = big.tile([P, G * P], F32, tag="gts")
        stage = big.tile([P, G * D], F32, tag="stage")
        for g in range(G):
            pt = psum.tile([P, P], F32, tag="pt")
            nc.tensor.transpose(pt[:], featsb[:, g * D : (g + 1) * D], eye[:])
            nc.vector.tensor_copy(gts[:, g * P : (g + 1) * P], pt[:])
        for g in range(G):
            po = psum2.tile([P, P], F32, tag="po")
            nc.tensor.matmul(
                po[:],
                lhsT=gts[:, g * P : (g + 1) * P],
                rhs=wt_sb[:],
                start=True,
                stop=True,
            )
            nc.scalar.copy(stage[:, g * D : (g + 1) * D], po[:])

        # own-j degree transposes, scale, and output -- per layer-half so the
        # first half drains while the second half's degree data is in flight
        degj = sml.tile([P, G], F32, tag="degj")
        d1 = sml.tile([P, G], F32, tag="d1")
        d2 = sml.tile([P, G], F32, tag="d2")
        prod = sml.tile([P, G], F32, tag="prod")
        sq = sml.tile([P, G], F32, tag="sq")
        rsc = sml.tile([P, G], F32, tag="rsc")
        LH = L // 2
        for h in range(2):
            gs = slice(h * G // 2, (h + 1) * G // 2)
            for t in range(T):
                ptj = psum.tile([P, 8], F32, tag="ptj")
                nc.tensor.transpose(
                    ptj[0:P, 0:4],
                    degj_raw[h][:, t * P : (t + 1) * P],
                    eye[0:4, 0:4],
                )
                nc.scalar.copy(
                    degj[:].rearrange("p (hh l t) -> p hh l t", hh=2, t=T)[:, h, :, t],
                    ptj[0:P, 0:4],
                )
            nc.vector.tensor_scalar(d1[:, gs], degs[:, gs], 1.0, None, OP.add)
            nc.vector.tensor_scalar(d2[:, gs], degj[:, gs], 1.0, None, OP.add)
            nc.vector.tensor_tensor(prod[:, gs], d1[:, gs], d2[:, gs], op=OP.mult)
            nc.scalar.activation(sq[:, gs], prod[:, gs], ACT.Sqrt)
            nc.vector.reciprocal(rsc[:, gs], sq[:, gs])
            for t in range(T):
                for l in range(h * LH, (h + 1) * LH):
                    g = l * T + t
                    nc.vector.tensor_scalar(
                        stage[:, g * D : (g + 1) * D],
                        stage[:, g * D : (g + 1) * D],
                        rsc[:, g : g + 1],
                        None,
                        OP.mult,
                    )
                nc.sync.dma_start(
                    out[t * P : (t + 1) * P, h * LH : (h + 1) * LH, :].rearrange(
                        "p l d -> p l d"
                    ),
                    stage[:].rearrange("p (l t d) -> p t l d", t=T, d=D)[
                        :, t, h * LH : (h + 1) * LH, :
                    ],
                )
        if debug:
            nc.sync.dma_start(o_degs.ap(), degs[:])
            nc.sync.dma_start(o_degj.ap(), degj[:])
            nc.sync.dma_start(o_rsc.ap(), rsc[:])

    nc.finalize()
    return nc


def shard_inputs(feature, W, adj, N=N_NODES, L=N_LAYERS, n_cores=N_CORES):
    """Host-side sharding/layout prep: bit-packing + layout transforms only."""
    JJ = N // n_cores
    T = JJ // P
    G = L * T
    WR = N // 32
    WD = JJ // 32
    NB = L * (N // P)
    DBLK = 64
    NI = G * P
    featg = np.ascontiguousarray(
        np.asarray(feature, dtype=np.float32).reshape(N * L, D)
    )
    wtr = np.ascontiguousarray(np.asarray(W, dtype=np.float32).T)
    a01 = np.asarray(adj) == 1  # [L, N(src), N(dest)] bool

    iotw = np.tile(np.arange(1, WR + 1, dtype=np.float32), (P, G)).reshape(P, G * WR)
    gl = np.repeat(np.arange(L), T).astype(np.float32)  # l per group
    gtt = np.tile(np.arange(T), L).astype(np.float32)  # t per group
    pp = np.arange(P, dtype=np.float32)[:, None]
    ladd = np.tile(gl, (P, 1)).astype(np.float32)
    # layer offset within the per-half degree tensors (deg_out1/deg_out2)
    lN = ((ladd % (L // 2)) * N).astype(np.float32)
    common = {
        "featg": featg,
        "wt": wtr,
        "iotw": iotw,
        "ladd": ladd,
        "lN": lN,
    }

    l_of_g = np.repeat(np.arange(L), T)
    t_of_g = np.tile(np.arange(T), L)
    r = np.arange(NI)
    g_of_r = r // P
    p_of_r = r % P

    in_maps = []
    for c in range(n_cores):
        j0 = c * JJ
        sl = a01[:, :, j0 : j0 + JJ]  # [L, N, JJ]
        # pbt image: [P, G*WR] i32; group (l,t), partition p -> row (l, t*128+p),
        # bits over source i (little bit order)
        bt = np.packbits(sl.transpose(0, 2, 1), axis=-1, bitorder="little")
        pbt = bt.reshape(L, T, P, WR * 4).transpose(2, 0, 1, 3).reshape(P, G * WR * 4)
        pbt = np.ascontiguousarray(pbt).view(np.int32)
        # pbd image: [P, NB*WD] i32; segment (l,b), partition p -> row (l, b*128+p),
        # bits over dest shard j
        bd = np.packbits(sl, axis=-1, bitorder="little")  # [L, N, JJ/8]
        pbd = (
            bd.reshape(L, N // P, P, WD * 4).transpose(2, 0, 1, 3).reshape(P, NB * WD * 4)
        )
        pbd = np.ascontiguousarray(pbd).view(np.int32)
        jgv = (j0 + gtt[None, :] * P + pp).astype(np.float32)
        # static wrapped idx for own-j degree blocks: flat r = g*128+p,
        # value = (l*N + j)//64; wrapped: tile[q, s] = val[r = s*16+q]
        djofs = ((np.arange(L, dtype=np.int32) % (L // 2)) * (N // JJ) + c).reshape(8, 1)
        in_maps.append({"pbt": pbt, "pbd": pbd, "jg": jgv, "djofs": djofs, **common})
    return in_maps


_NC_CACHE = {}
LAST_RESULT = None


def kernel(feature, W, adj):
    global LAST_RESULT
    _install_ntff_hook()
    from concourse.bass_utils import run_bass_kernel_spmd

    feature = np.asarray(feature)
    W = np.asarray(W)
    adj = np.asarray(adj)
    N, L, _ = feature.shape
    key = (N, L)
    if key not in _NC_CACHE:
        _NC_CACHE[key] = build_kernel(N=N, L=L)
    nc = _NC_CACHE[key]

    in_maps = shard_inputs(feature, W, adj, N=N, L=L)
    res = run_bass_kernel_spmd(nc, in_maps, core_ids=list(range(N_CORES)))
    LAST_RESULT = res
    return np.concatenate([res.results[c]["out"] for c in range(N_CORES)], axis=0)
